# revision 16
# baseline (speedup 1.0000x reference)
"""Bahdanau attention on 8 Trainium2 NeuronCores.

Problem shapes (hardcoded): B=32, S=4096, D=512, U=128.
Sharding: data-parallel over batch - 4 batches per core, weights replicated.

Per core, per batch b (values in fp16 on-chip, fp32 accumulation):
  vh = fp16(values[b])                 (cast during DMA load, SWDGE)
  vT tiles = vh.T                      (PE transpose, fp16)
  p_T[u,s] = W2.T @ vT                 (PE fp16, N=512, fp32 PSUM)
  t_T = tanh(p_T + q_proj[b]+b1+b2)    (ScalarE, per-partition bias, fp16 out)
  score_row[1,s] = V.T @ t_T           (PE fp16, fp32 PSUM)
  rows -> DRAM scratch -> [32,128] -> PE transpose -> score columns [128,32]
  e = exp(score) (+row sums)  (no max subtraction: |score| <= ||V||_1 ~ 10)
  Z = sum(e) (PE ones-matvec), w = e/Z (fp16 for the context pass)
  ctx = sum_s w[s] * values[b,s,:]     (PE accumulating fp16 matvecs)
softmax is shift-invariant so bv drops out of both outputs.
"""

import numpy as np

import concourse.bass as bass
import concourse.mybir as mybir
import concourse.tile as tile
from concourse.bass_utils import run_bass_kernel_spmd
from concourse.masks import make_identity
from concourse.tile import add_dep_helper
from concourse.vector_clock import ScopedClock

F32 = mybir.dt.float32
F16 = mybir.dt.float16
AF = mybir.ActivationFunctionType
ALU = mybir.AluOpType

B, S, D, U = 32, 4096, 512, 128
NCORES = 8
BPC = B // NCORES          # batches per core = 4
NJ = S // 128              # 32 s-tiles of 128
NK = D // 128              # 4 d-chunks of 128
NM = S // 512              # 8 macro s-chunks of 512


# --- workaround (1/2): this container's walrus codegen accepts at most ONE
# sync-wait per instruction (two on EventSemaphore), but this bass/tile
# emits instructions carrying several. Post-process the serialized BIR:
# keep the first wait and move the surplus onto EventSemaphore carriers
# inserted just before it on the same engine.
_fix_counter = [0]


def _fix_bir_waits(data):
    import json

    d = json.loads(data)
    for fn in d.get("functions", []):
        for bb in fn.get("blocks", []):
            insts = bb.get("instructions", [])
            out = []
            for inst in insts:
                si = inst.get("sync_info") or {}
                ow = si.get("on_wait") or []
                if len(ow) > 1:
                    keep = ow[:1]
                    surplus = ow[1:]
                    insert_at = len(out)
                    if (
                        inst.get("opcode") == "Matmult"
                        and out
                        and out[-1].get("opcode") == "Ldweights"
                        and out[-1].get("engine") == inst.get("engine")
                    ):
                        insert_at = len(out) - 1
                    carriers = []
                    for i in range(0, len(surplus), 2):
                        _fix_counter[0] += 1
                        carriers.append(
                            {
                                "engine": inst["engine"],
                                "ins": [],
                                "name": f"I-waitfix-{_fix_counter[0]}",
                                "opcode": "EventSemaphore",
                                "outs": [],
                                "sync_info": {
                                    "on_update": [],
                                    "on_wait": surplus[i : i + 2],
                                },
                            }
                        )
                    out[insert_at:insert_at] = carriers
                    si["on_wait"] = keep
                out.append(inst)
            bb["instructions"] = out
    return json.dumps(d).encode()


_orig_to_json_bytes = bass.Bass.to_json_bytes


def _patched_to_json_bytes(self, *a, **kw):
    return _fix_bir_waits(_orig_to_json_bytes(self, *a, **kw))


bass.Bass.to_json_bytes = _patched_to_json_bytes


# --- workaround (2/2): TileContext exit puts one wait per outstanding DMA
# queue on a single Drain; split across extra Drains.
def _split_drain_and_barrier(self, tick_clock, wait_clock):
    drain_inst = self.nc.sync.drain()
    wait_clock.add_sem_waits(
        drain_inst.ins, ScopedClock({None: tick_clock.global_clock})
    )
    waits = list(drain_inst.ins.sync_info.on_wait)
    if len(waits) > 1:
        drain_inst.ins.sync_info.on_wait = waits[:1]
        by_name = {h.name: h for h in self.sems.allocated().values()}
        for w in waits[1:]:
            extra = self.nc.sync.drain()
            extra.wait_op(by_name[w.ant_name], w.wait_value, "sem-ge")
    self.nc.all_engine_barrier()
    assert self.sems is not None
    popped = self.nc._tile_sem_poison_stack.pop()
    assert popped is self._sem_poison
    self.nc.clear_and_free_semaphores(list(self.sems.allocated().values()))
    self.nc.all_engine_barrier()


tile.TileContext._drain_and_barrier = _split_drain_and_barrier


def build_kernel():
    nc = bass.Bass("TRN2", target_bir_lowering=False, debug=False)

    query = nc.dram_tensor("query", [BPC, D], F32, kind="ExternalInput").ap()
    values = nc.dram_tensor("values", [BPC, S, D], F32, kind="ExternalInput").ap()
    W1 = nc.dram_tensor("W1", [D, U], F32, kind="ExternalInput").ap()
    b1 = nc.dram_tensor("b1", [U, 1], F32, kind="ExternalInput").ap()
    W2 = nc.dram_tensor("W2", [D, U], F32, kind="ExternalInput").ap()
    b2 = nc.dram_tensor("b2", [U, 1], F32, kind="ExternalInput").ap()
    V = nc.dram_tensor("V", [U, 1], F32, kind="ExternalInput").ap()
    ctx_out = nc.dram_tensor("ctx", [BPC, D], F32, kind="ExternalOutput").ap()
    aw_out = nc.dram_tensor("aw", [BPC, S], F32, kind="ExternalOutput").ap()
    # score-row bounce buffer (columnarization); ignored by the host
    srows = nc.dram_tensor("srows", [BPC, NM, 512], F32, kind="ExternalOutput").ap()

    with tile.TileContext(nc) as tc:
        with (
            tc.tile_pool(name="const", bufs=1) as const,
            tc.tile_pool(name="vh", bufs=24) as vh_pool,
            tc.tile_pool(name="vt", bufs=3) as vt_pool,
            tc.tile_pool(name="tt", bufs=3) as tt_pool,
            tc.tile_pool(name="small", bufs=2) as small,
            tc.tile_pool(name="vtp", bufs=2, space="PSUM") as vtpsum,
            tc.tile_pool(name="ppp", bufs=2, space="PSUM") as ppsum,
            tc.tile_pool(name="srp", bufs=1, space="PSUM") as srpsum,
            tc.tile_pool(name="cxp", bufs=1, space="PSUM") as ctxpsum,
        ):
            # ---- constants ----
            W1sb = const.tile([128, NK, U], F32)
            nc.sync.dma_start(out=W1sb, in_=W1.rearrange("(k p) u -> p k u", p=128))
            W2h = const.tile([128, NK, U], F16)
            nc.gpsimd.dma_start(out=W2h, in_=W2.rearrange("(k p) u -> p k u", p=128))
            b1sb = const.tile([128, 1], F32)
            nc.sync.dma_start(out=b1sb, in_=b1)
            b2sb = const.tile([128, 1], F32)
            nc.sync.dma_start(out=b2sb, in_=b2)
            Vsb = const.tile([128, 1], F32)
            nc.sync.dma_start(out=Vsb, in_=V)
            Vh = const.tile([128, 1], F16)
            nc.vector.tensor_copy(Vh, Vsb)
            ident = const.tile([128, 128], F32)
            make_identity(nc, ident)
            identh = const.tile([128, 128], F16)
            nc.vector.tensor_copy(identh, ident)
            ones = const.tile([128, 128], F32)
            nc.vector.memset(ones, 1.0)

            # ---- q_proj (per core, all 4 batches at once) ----
            qT = const.tile([128, NK, BPC], F32)
            for k in range(NK):
                nc.gpsimd.dma_start(
                    out=qT[:, k, :],
                    in_=query[:, k * 128 : (k + 1) * 128].rearrange("b p -> p b"),
                )
            qp_t = ppsum.tile([128, 512], F32, tag="pp")
            qp = qp_t[:, :BPC]
            for k in range(NK):
                nc.tensor.matmul(
                    qp, W1sb[:, k, :], qT[:, k, :], start=(k == 0), stop=(k == NK - 1)
                )
            qpT = const.tile([128, BPC], F32)
            # q_proj + b1 + b2 (fold both biases into the tanh bias)
            nc.vector.tensor_scalar_add(qpT, qp, b1sb)
            nc.vector.tensor_scalar_add(qpT, qpT, b2sb)

            # ---- batches (software-pipelined: ctx/aw of batch b-1 are
            # emitted between the score phase and softmax of batch b, so the
            # PE stream never stalls on the softmax serial chain) ----
            state = {}

            def score_phase(b):
                vb = []
                for q in range(8):
                    t = vh_pool.tile([128, 4, D], F16, tag="vh")
                    nc.gpsimd.dma_start(
                        out=t,
                        in_=values[b, q * 512 : (q + 1) * 512, :].rearrange(
                            "(j p) d -> p j d", p=128
                        ),
                    )
                    vb.append(t)

                row_writes = []
                for m in range(NM):
                    qtile = vb[m]
                    jbase = 0
                    vtp = vtpsum.tile([128, NK, 512], F16)
                    for k in range(NK):
                        for j4 in range(4):
                            nc.tensor.transpose(
                                vtp[:, k, j4 * 128 : (j4 + 1) * 128],
                                qtile[:, j4, k * 128 : (k + 1) * 128],
                                identh,
                            )
                    vt = vt_pool.tile([128, NK, 512], F16)
                    for k in range(NK):
                        if k < 2:
                            nc.vector.tensor_copy(vt[:, k, :], vtp[:, k, :])
                        else:
                            nc.scalar.copy(vt[:, k, :], vtp[:, k, :])
                    pp = ppsum.tile([128, 512], F32, tag="pp")
                    for k in range(NK):
                        nc.tensor.matmul(
                            pp,
                            W2h[:, k, :],
                            vt[:, k, :],
                            start=(k == 0),
                            stop=(k == NK - 1),
                        )
                    tT = tt_pool.tile([128, 512], F16)
                    nc.scalar.activation(
                        tT, pp, AF.Tanh, bias=qpT[:, b : b + 1], scale=1.0
                    )
                    srow = srpsum.tile([128, 512], F32, tag="sr")
                    nc.tensor.matmul(srow[0:1, :], Vh, tT, start=True, stop=True)
                    row_sb = small.tile([1, 512], F32, tag="rowsb")
                    nc.vector.tensor_copy(row_sb, srow[0:1, :])
                    w_ins = nc.sync.dma_start(out=srows[b, m : m + 1, :], in_=row_sb)
                    row_writes.append(w_ins)
                state[b] = {"vb": vb, "row_writes": row_writes}

            def softmax_phase(b):
                st = state[b]
                e_sb = small.tile([128, NJ], F32, tag="esb")
                e16s = []
                rss = []
                for h in range(2):
                    rows32 = small.tile([16, 128], F32, tag=f"rows32{h}")
                    r_ins = nc.sync.dma_start(
                        out=rows32,
                        in_=srows[b, 4 * h : 4 * h + 4].rearrange(
                            "a (c f) -> (a c) f", f=128
                        ),
                    )
                    for w_ins in st["row_writes"][4 * h : 4 * h + 4]:
                        add_dep_helper(r_ins.ins, w_ins.ins, reason="scratch RAW")
                    sc_p = srpsum.tile([128, 512], F32, tag="sr")
                    nc.tensor.matmul(
                        sc_p[:, :16], rows32, ident[:16, :16], is_transpose=True,
                        start=True, stop=True,
                    )
                    rs = small.tile([128, 1], F32, tag=f"rs{h}")
                    nc.scalar.activation(
                        e_sb[:, 16 * h : 16 * h + 16], sc_p[:, :16], AF.Exp,
                        accum_out=rs,
                    )
                    e16 = small.tile([128, 16], F16, tag=f"e16{h}")
                    nc.vector.tensor_copy(e16, e_sb[:, 16 * h : 16 * h + 16])
                    e16s.append(e16)
                    rss.append(rs)
                mp = ppsum.tile([128, 512], F32, tag="pp")
                nc.tensor.matmul(mp[0:1, 0:1], rss[0], ones[:, 0:1], start=True, stop=False)
                nc.tensor.matmul(mp[0:1, 0:1], rss[1], ones[:, 0:1], start=False, stop=True)
                invZ = small.tile([1, 1], F32, tag="invZ")
                nc.vector.reciprocal(invZ, mp[0:1, 0:1])
                nc.tensor.matmul(mp[:, 4:5], ones[0:1, :], invZ, start=True, stop=True)
                inv_p = small.tile([128, 1], F32, tag="invp")
                nc.vector.tensor_copy(inv_p, mp[:, 4:5])
                st.update(e_sb=e_sb, e16s=e16s, invZ=invZ, inv_p=inv_p)

            def ctx_phase(b):
                st = state[b]
                vb, e_sb, e16s = st["vb"], st["e_sb"], st["e16s"]
                # context (unnormalized e; scaled by 1/Z at the end)
                cp = ctxpsum.tile([128, 512], F32)
                for j in range(NJ):
                    nc.tensor.matmul(
                        cp[0:1, :],
                        e16s[j // 16][:, j % 16 : j % 16 + 1],
                        vb[j // 4][:, j % 4, :],
                        start=(j == 0),
                        stop=(j == NJ - 1),
                    )
                ctx_sb = small.tile([1, 512], F32, tag="ctxsb")
                nc.vector.tensor_scalar_mul(ctx_sb, cp[0:1, :], st["invZ"])
                nc.sync.dma_start(out=ctx_out[b : b + 1, :], in_=ctx_sb)
                # attention-weights output
                wt_p = srpsum.tile([128, 512], F32, tag="sr")
                nc.tensor.transpose(wt_p[:NJ, :128], e_sb, ident)
                wt_sb = small.tile([NJ, 128], F32, tag="wtsb")
                nc.vector.tensor_scalar_mul(wt_sb, wt_p[:NJ, :128], st["inv_p"][:NJ, :])
                nc.sync.dma_start(
                    out=aw_out[b].rearrange("(j f) -> j f", f=128), in_=wt_sb
                )
                del state[b]

            score_phase(0)
            score_phase(1)
            softmax_phase(0)
            ctx_phase(0)
            score_phase(2)
            softmax_phase(1)
            ctx_phase(1)
            score_phase(3)
            softmax_phase(2)
            ctx_phase(2)
            softmax_phase(3)
            ctx_phase(3)

    return nc


_NC_CACHE = {}


def kernel(query, values, W1, b1, W2, b2, V, bv):
    query = np.ascontiguousarray(np.asarray(query, dtype=np.float32))
    values = np.ascontiguousarray(np.asarray(values, dtype=np.float32))
    W1 = np.ascontiguousarray(np.asarray(W1, dtype=np.float32))
    b1 = np.asarray(b1, dtype=np.float32).reshape(U, 1)
    W2 = np.ascontiguousarray(np.asarray(W2, dtype=np.float32))
    b2 = np.asarray(b2, dtype=np.float32).reshape(U, 1)
    V = np.ascontiguousarray(np.asarray(V, dtype=np.float32).reshape(U, 1))

    if "nc" not in _NC_CACHE:
        _NC_CACHE["nc"] = build_kernel()
    nc = _NC_CACHE["nc"]

    in_maps = []
    for c in range(NCORES):
        sl = slice(c * BPC, (c + 1) * BPC)
        in_maps.append(
            {
                "query": np.ascontiguousarray(query[sl]),
                "values": np.ascontiguousarray(values[sl]),
                "W1": W1,
                "b1": b1,
                "W2": W2,
                "b2": b2,
                "V": V,
            }
        )

    res = run_bass_kernel_spmd(nc, in_maps, core_ids=list(range(NCORES)))
    ctx = np.concatenate([res.results[c]["ctx"] for c in range(NCORES)], axis=0)
    aw = np.concatenate([res.results[c]["aw"] for c in range(NCORES)], axis=0)
    return ctx, aw.reshape(B, S, 1)


# revision 17
# speedup vs baseline: 1.0254x; 1.0254x over previous
"""Bahdanau attention on 8 Trainium2 NeuronCores.

Problem shapes (hardcoded): B=32, S=4096, D=512, U=128.
Sharding: data-parallel over batch - 4 batches per core, weights replicated.

Per core, per batch b (values in fp16 on-chip, fp32 accumulation):
  vh = fp16(values[b])                 (cast during DMA load, SWDGE)
  vT tiles = vh.T                      (PE transpose, fp16)
  p_T[u,s] = W2.T @ vT                 (PE fp16, N=512, fp32 PSUM)
  t_T = tanh(p_T + q_proj[b]+b1+b2)    (ScalarE, per-partition bias, fp16 out)
  score_row[1,s] = V.T @ t_T           (PE fp16, fp32 PSUM)
  rows -> DRAM scratch -> [32,128] -> PE transpose -> score columns [128,32]
  e = exp(score) (+row sums)  (no max subtraction: |score| <= ||V||_1 ~ 10)
  Z = sum(e) (PE ones-matvec), w = e/Z (fp16 for the context pass)
  ctx = sum_s w[s] * values[b,s,:]     (PE accumulating fp16 matvecs)
softmax is shift-invariant so bv drops out of both outputs.
"""

import numpy as np

import concourse.bass as bass
import concourse.mybir as mybir
import concourse.tile as tile
from concourse.bass_utils import run_bass_kernel_spmd
from concourse.masks import make_identity
from concourse.tile import add_dep_helper
from concourse.vector_clock import ScopedClock

F32 = mybir.dt.float32
F16 = mybir.dt.float16
AF = mybir.ActivationFunctionType
ALU = mybir.AluOpType

B, S, D, U = 32, 4096, 512, 128
NCORES = 8
BPC = B // NCORES          # batches per core = 4
NJ = S // 128              # 32 s-tiles of 128
NK = D // 128              # 4 d-chunks of 128
NM = S // 512              # 8 macro s-chunks of 512


# --- workaround (1/2): this container's walrus codegen accepts at most ONE
# sync-wait per instruction (two on EventSemaphore), but this bass/tile
# emits instructions carrying several. Post-process the serialized BIR:
# keep the first wait and move the surplus onto EventSemaphore carriers
# inserted just before it on the same engine.
_fix_counter = [0]


def _fix_bir_waits(data):
    import json

    d = json.loads(data)
    for fn in d.get("functions", []):
        for bb in fn.get("blocks", []):
            insts = bb.get("instructions", [])
            out = []
            for inst in insts:
                si = inst.get("sync_info") or {}
                ow = si.get("on_wait") or []
                if len(ow) > 1:
                    keep = ow[:1]
                    surplus = ow[1:]
                    insert_at = len(out)
                    if (
                        inst.get("opcode") == "Matmult"
                        and out
                        and out[-1].get("opcode") == "Ldweights"
                        and out[-1].get("engine") == inst.get("engine")
                    ):
                        insert_at = len(out) - 1
                    carriers = []
                    for i in range(0, len(surplus), 2):
                        _fix_counter[0] += 1
                        carriers.append(
                            {
                                "engine": inst["engine"],
                                "ins": [],
                                "name": f"I-waitfix-{_fix_counter[0]}",
                                "opcode": "EventSemaphore",
                                "outs": [],
                                "sync_info": {
                                    "on_update": [],
                                    "on_wait": surplus[i : i + 2],
                                },
                            }
                        )
                    out[insert_at:insert_at] = carriers
                    si["on_wait"] = keep
                out.append(inst)
            bb["instructions"] = out
    return json.dumps(d).encode()


_orig_to_json_bytes = bass.Bass.to_json_bytes


def _patched_to_json_bytes(self, *a, **kw):
    return _fix_bir_waits(_orig_to_json_bytes(self, *a, **kw))


bass.Bass.to_json_bytes = _patched_to_json_bytes


# --- workaround (2/2): TileContext exit puts one wait per outstanding DMA
# queue on a single Drain; split across extra Drains.
def _split_drain_and_barrier(self, tick_clock, wait_clock):
    drain_inst = self.nc.sync.drain()
    wait_clock.add_sem_waits(
        drain_inst.ins, ScopedClock({None: tick_clock.global_clock})
    )
    waits = list(drain_inst.ins.sync_info.on_wait)
    if len(waits) > 1:
        drain_inst.ins.sync_info.on_wait = waits[:1]
        by_name = {h.name: h for h in self.sems.allocated().values()}
        for w in waits[1:]:
            extra = self.nc.sync.drain()
            extra.wait_op(by_name[w.ant_name], w.wait_value, "sem-ge")
    self.nc.all_engine_barrier()
    assert self.sems is not None
    popped = self.nc._tile_sem_poison_stack.pop()
    assert popped is self._sem_poison
    self.nc.clear_and_free_semaphores(list(self.sems.allocated().values()))
    self.nc.all_engine_barrier()


tile.TileContext._drain_and_barrier = _split_drain_and_barrier


def build_kernel():
    nc = bass.Bass("TRN2", target_bir_lowering=False, debug=False)

    query = nc.dram_tensor("query", [BPC, D], F32, kind="ExternalInput").ap()
    values = nc.dram_tensor("values", [BPC, S, D], F32, kind="ExternalInput").ap()
    W1 = nc.dram_tensor("W1", [D, U], F32, kind="ExternalInput").ap()
    b1 = nc.dram_tensor("b1", [U, 1], F32, kind="ExternalInput").ap()
    W2 = nc.dram_tensor("W2", [D, U], F32, kind="ExternalInput").ap()
    b2 = nc.dram_tensor("b2", [U, 1], F32, kind="ExternalInput").ap()
    V = nc.dram_tensor("V", [U, 1], F32, kind="ExternalInput").ap()
    ctx_out = nc.dram_tensor("ctx", [BPC, D], F32, kind="ExternalOutput").ap()
    aw_out = nc.dram_tensor("aw", [BPC, S], F32, kind="ExternalOutput").ap()
    # score-row bounce buffer (columnarization); ignored by the host
    srows = nc.dram_tensor("srows", [BPC, NM, 512], F32, kind="ExternalOutput").ap()

    with tile.TileContext(nc) as tc:
        with (
            tc.tile_pool(name="const", bufs=1) as const,
            tc.tile_pool(name="vh", bufs=24) as vh_pool,
            tc.tile_pool(name="vt", bufs=3) as vt_pool,
            tc.tile_pool(name="tt", bufs=3) as tt_pool,
            tc.tile_pool(name="small", bufs=2) as small,
            tc.tile_pool(name="vtp", bufs=2, space="PSUM") as vtpsum,
            tc.tile_pool(name="ppp", bufs=2, space="PSUM") as ppsum,
            tc.tile_pool(name="srp", bufs=1, space="PSUM") as srpsum,
            tc.tile_pool(name="cxp", bufs=1, space="PSUM") as ctxpsum,
        ):
            # ---- constants ----
            W1sb = const.tile([128, NK, U], F32)
            nc.sync.dma_start(out=W1sb, in_=W1.rearrange("(k p) u -> p k u", p=128))
            W2h = const.tile([128, NK, U], F16)
            nc.gpsimd.dma_start(out=W2h, in_=W2.rearrange("(k p) u -> p k u", p=128))
            b1sb = const.tile([128, 1], F32)
            nc.sync.dma_start(out=b1sb, in_=b1)
            b2sb = const.tile([128, 1], F32)
            nc.sync.dma_start(out=b2sb, in_=b2)
            Vsb = const.tile([128, 1], F32)
            nc.sync.dma_start(out=Vsb, in_=V)
            Vh = const.tile([128, 1], F16)
            nc.vector.tensor_copy(Vh, Vsb)
            ident = const.tile([128, 128], F32)
            make_identity(nc, ident)
            identh = const.tile([128, 128], F16)
            nc.vector.tensor_copy(identh, ident)
            ones = const.tile([128, 128], F32)
            nc.vector.memset(ones, 1.0)

            # ---- q_proj (per core, all 4 batches at once) ----
            qT = const.tile([128, NK, BPC], F32)
            for k in range(NK):
                nc.gpsimd.dma_start(
                    out=qT[:, k, :],
                    in_=query[:, k * 128 : (k + 1) * 128].rearrange("b p -> p b"),
                )
            qp_t = ppsum.tile([128, 512], F32, tag="pp")
            qp = qp_t[:, :BPC]
            for k in range(NK):
                nc.tensor.matmul(
                    qp, W1sb[:, k, :], qT[:, k, :], start=(k == 0), stop=(k == NK - 1)
                )
            qpT = const.tile([128, BPC], F32)
            # q_proj + b1 + b2 (fold both biases into the tanh bias)
            nc.vector.tensor_scalar_add(qpT, qp, b1sb)
            nc.vector.tensor_scalar_add(qpT, qpT, b2sb)

            # ---- batches (software-pipelined: ctx/aw of batch b-1 are
            # emitted between the score phase and softmax of batch b, so the
            # PE stream never stalls on the softmax serial chain) ----
            state = {}

            def score_phase(b):
                vb = []
                for q in range(8):
                    t = vh_pool.tile([128, 4, D], F16, tag="vh")
                    nc.gpsimd.dma_start(
                        out=t,
                        in_=values[b, q * 512 : (q + 1) * 512, :].rearrange(
                            "(j p) d -> p j d", p=128
                        ),
                    )
                    vb.append(t)

                row_writes = []
                for m in range(NM):
                    qtile = vb[m]
                    jbase = 0
                    vtp = vtpsum.tile([128, NK, 512], F16)
                    for k in range(NK):
                        for j4 in range(4):
                            nc.tensor.transpose(
                                vtp[:, k, j4 * 128 : (j4 + 1) * 128],
                                qtile[:, j4, k * 128 : (k + 1) * 128],
                                identh,
                            )
                    vt = vt_pool.tile([128, NK, 512], F16)
                    for k in range(NK):
                        if k < 2:
                            nc.vector.tensor_copy(vt[:, k, :], vtp[:, k, :])
                        else:
                            nc.scalar.copy(vt[:, k, :], vtp[:, k, :])
                    pp = ppsum.tile([128, 512], F32, tag="pp")
                    for k in range(NK):
                        nc.tensor.matmul(
                            pp,
                            W2h[:, k, :],
                            vt[:, k, :],
                            start=(k == 0),
                            stop=(k == NK - 1),
                        )
                    tT = tt_pool.tile([128, 512], F16)
                    nc.scalar.activation(
                        tT, pp, AF.Tanh, bias=qpT[:, b : b + 1], scale=1.0
                    )
                    srow = srpsum.tile([128, 512], F32, tag="sr")
                    nc.tensor.matmul(srow[0:1, :], Vh, tT, start=True, stop=True)
                    row_sb = small.tile([1, 512], F32, tag="rowsb")
                    nc.vector.tensor_copy(row_sb, srow[0:1, :])
                    w_ins = nc.sync.dma_start(out=srows[b, m : m + 1, :], in_=row_sb)
                    row_writes.append(w_ins)
                state[b] = {"vb": vb, "row_writes": row_writes}

            def tail_phase(b):
                st = state[b]
                vb = st["vb"]
                e_sb = small.tile([128, NJ], F32, tag="esb")
                rss = []
                cp = ctxpsum.tile([128, 512], F32)
                for h in range(2):
                    rows32 = small.tile([16, 128], F32, tag=f"rows32{h}")
                    r_ins = nc.sync.dma_start(
                        out=rows32,
                        in_=srows[b, 4 * h : 4 * h + 4].rearrange(
                            "a (c f) -> (a c) f", f=128
                        ),
                    )
                    for w_ins in st["row_writes"][4 * h : 4 * h + 4]:
                        add_dep_helper(r_ins.ins, w_ins.ins, reason="scratch RAW")
                    sc_p = srpsum.tile([128, 512], F32, tag="sr")
                    nc.tensor.matmul(
                        sc_p[:, :16], rows32, ident[:16, :16], is_transpose=True,
                        start=True, stop=True,
                    )
                    rs = small.tile([128, 1], F32, tag=f"rs{h}")
                    nc.scalar.activation(
                        e_sb[:, 16 * h : 16 * h + 16], sc_p[:, :16], AF.Exp,
                        accum_out=rs,
                    )
                    rss.append(rs)
                    e16 = small.tile([128, 16], F16, tag=f"e16{h}")
                    nc.vector.tensor_copy(e16, e_sb[:, 16 * h : 16 * h + 16])
                    # context half (unnormalized e; scaled by 1/Z at the end)
                    for jj in range(16):
                        j = 16 * h + jj
                        nc.tensor.matmul(
                            cp[0:1, :],
                            e16[:, jj : jj + 1],
                            vb[j // 4][:, j % 4, :],
                            start=(j == 0),
                            stop=(j == NJ - 1),
                        )
                mp = ppsum.tile([128, 512], F32, tag="pp")
                nc.tensor.matmul(mp[0:1, 0:1], rss[0], ones[:, 0:1], start=True, stop=False)
                nc.tensor.matmul(mp[0:1, 0:1], rss[1], ones[:, 0:1], start=False, stop=True)
                invZ = small.tile([1, 1], F32, tag="invZ")
                nc.vector.reciprocal(invZ, mp[0:1, 0:1])
                nc.tensor.matmul(mp[:, 4:5], ones[0:1, :], invZ, start=True, stop=True)
                inv_p = small.tile([128, 1], F32, tag="invp")
                nc.vector.tensor_copy(inv_p, mp[:, 4:5])
                ctx_sb = small.tile([1, 512], F32, tag="ctxsb")
                nc.vector.tensor_scalar_mul(ctx_sb, cp[0:1, :], invZ)
                nc.sync.dma_start(out=ctx_out[b : b + 1, :], in_=ctx_sb)
                wt_p = srpsum.tile([128, 512], F32, tag="sr")
                nc.tensor.transpose(wt_p[:NJ, :128], e_sb, ident)
                wt_sb = small.tile([NJ, 128], F32, tag="wtsb")
                nc.vector.tensor_scalar_mul(wt_sb, wt_p[:NJ, :128], inv_p[:NJ, :])
                nc.sync.dma_start(
                    out=aw_out[b].rearrange("(j f) -> j f", f=128), in_=wt_sb
                )
                del state[b]

            score_phase(0)
            score_phase(1)
            tail_phase(0)
            score_phase(2)
            tail_phase(1)
            score_phase(3)
            tail_phase(2)
            tail_phase(3)

    return nc


_NC_CACHE = {}


def kernel(query, values, W1, b1, W2, b2, V, bv):
    query = np.ascontiguousarray(np.asarray(query, dtype=np.float32))
    values = np.ascontiguousarray(np.asarray(values, dtype=np.float32))
    W1 = np.ascontiguousarray(np.asarray(W1, dtype=np.float32))
    b1 = np.asarray(b1, dtype=np.float32).reshape(U, 1)
    W2 = np.ascontiguousarray(np.asarray(W2, dtype=np.float32))
    b2 = np.asarray(b2, dtype=np.float32).reshape(U, 1)
    V = np.ascontiguousarray(np.asarray(V, dtype=np.float32).reshape(U, 1))

    if "nc" not in _NC_CACHE:
        _NC_CACHE["nc"] = build_kernel()
    nc = _NC_CACHE["nc"]

    in_maps = []
    for c in range(NCORES):
        sl = slice(c * BPC, (c + 1) * BPC)
        in_maps.append(
            {
                "query": np.ascontiguousarray(query[sl]),
                "values": np.ascontiguousarray(values[sl]),
                "W1": W1,
                "b1": b1,
                "W2": W2,
                "b2": b2,
                "V": V,
            }
        )

    res = run_bass_kernel_spmd(nc, in_maps, core_ids=list(range(NCORES)))
    ctx = np.concatenate([res.results[c]["ctx"] for c in range(NCORES)], axis=0)
    aw = np.concatenate([res.results[c]["aw"] for c in range(NCORES)], axis=0)
    return ctx, aw.reshape(B, S, 1)


# revision 18
# speedup vs baseline: 1.0937x; 1.0666x over previous
"""Bahdanau attention on 8 Trainium2 NeuronCores.

Problem shapes (hardcoded): B=32, S=4096, D=512, U=128.
Sharding: data-parallel over batch - 4 batches per core, weights replicated.

Per core, per batch b (values in fp16 on-chip, fp32 accumulation):
  vh = fp16(values[b])                 (cast during DMA load, SWDGE)
  vT tiles = vh.T                      (PE transpose, fp16)
  p_T[u,s] = W2.T @ vT                 (PE fp16, N=512, fp32 PSUM)
  t_T = tanh(p_T + q_proj[b]+b1+b2)    (ScalarE, per-partition bias, fp16 out)
  score_row[1,s] = V.T @ t_T           (PE fp16, fp32 PSUM)
  rows -> DRAM scratch -> [32,128] -> PE transpose -> score columns [128,32]
  e = exp(score) (+row sums)  (no max subtraction: |score| <= ||V||_1 ~ 10)
  Z = sum(e) (PE ones-matvec), w = e/Z (fp16 for the context pass)
  ctx = sum_s w[s] * values[b,s,:]     (PE accumulating fp16 matvecs)
softmax is shift-invariant so bv drops out of both outputs.
"""

import numpy as np

import concourse.bass as bass
import concourse.mybir as mybir
import concourse.tile as tile
from concourse.bass_utils import run_bass_kernel_spmd
from concourse.masks import make_identity
from concourse.tile import add_dep_helper
from concourse.vector_clock import ScopedClock

F32 = mybir.dt.float32
F16 = mybir.dt.float16
AF = mybir.ActivationFunctionType
ALU = mybir.AluOpType

B, S, D, U = 32, 4096, 512, 128
NCORES = 8
BPC = B // NCORES          # batches per core = 4
NJ = S // 128              # 32 s-tiles of 128
NK = D // 128              # 4 d-chunks of 128
NM = S // 512              # 8 macro s-chunks of 512


# --- workaround (1/2): this container's walrus codegen accepts at most ONE
# sync-wait per instruction (two on EventSemaphore), but this bass/tile
# emits instructions carrying several. Post-process the serialized BIR:
# keep the first wait and move the surplus onto EventSemaphore carriers
# inserted just before it on the same engine.
_fix_counter = [0]


def _fix_bir_waits(data):
    import json

    d = json.loads(data)
    for fn in d.get("functions", []):
        for bb in fn.get("blocks", []):
            insts = bb.get("instructions", [])
            out = []
            for inst in insts:
                si = inst.get("sync_info") or {}
                ow = si.get("on_wait") or []
                if len(ow) > 1:
                    keep = ow[:1]
                    surplus = ow[1:]
                    insert_at = len(out)
                    if (
                        inst.get("opcode") == "Matmult"
                        and out
                        and out[-1].get("opcode") == "Ldweights"
                        and out[-1].get("engine") == inst.get("engine")
                    ):
                        insert_at = len(out) - 1
                    carriers = []
                    for i in range(0, len(surplus), 2):
                        _fix_counter[0] += 1
                        carriers.append(
                            {
                                "engine": inst["engine"],
                                "ins": [],
                                "name": f"I-waitfix-{_fix_counter[0]}",
                                "opcode": "EventSemaphore",
                                "outs": [],
                                "sync_info": {
                                    "on_update": [],
                                    "on_wait": surplus[i : i + 2],
                                },
                            }
                        )
                    out[insert_at:insert_at] = carriers
                    si["on_wait"] = keep
                out.append(inst)
            bb["instructions"] = out
    return json.dumps(d).encode()


_orig_to_json_bytes = bass.Bass.to_json_bytes


def _patched_to_json_bytes(self, *a, **kw):
    return _fix_bir_waits(_orig_to_json_bytes(self, *a, **kw))


bass.Bass.to_json_bytes = _patched_to_json_bytes


# --- workaround (2/2): TileContext exit puts one wait per outstanding DMA
# queue on a single Drain; split across extra Drains.
def _split_drain_and_barrier(self, tick_clock, wait_clock):
    drain_inst = self.nc.sync.drain()
    wait_clock.add_sem_waits(
        drain_inst.ins, ScopedClock({None: tick_clock.global_clock})
    )
    waits = list(drain_inst.ins.sync_info.on_wait)
    if len(waits) > 1:
        drain_inst.ins.sync_info.on_wait = waits[:1]
        by_name = {h.name: h for h in self.sems.allocated().values()}
        for w in waits[1:]:
            extra = self.nc.sync.drain()
            extra.wait_op(by_name[w.ant_name], w.wait_value, "sem-ge")
    self.nc.all_engine_barrier()
    assert self.sems is not None
    popped = self.nc._tile_sem_poison_stack.pop()
    assert popped is self._sem_poison
    self.nc.clear_and_free_semaphores(list(self.sems.allocated().values()))
    self.nc.all_engine_barrier()


tile.TileContext._drain_and_barrier = _split_drain_and_barrier


def build_kernel():
    nc = bass.Bass("TRN2", target_bir_lowering=False, debug=False)

    query = nc.dram_tensor("query", [BPC, D], F32, kind="ExternalInput").ap()
    values = nc.dram_tensor("values", [BPC, S, D], F32, kind="ExternalInput").ap()
    W1 = nc.dram_tensor("W1", [D, U], F32, kind="ExternalInput").ap()
    b1 = nc.dram_tensor("b1", [U, 1], F32, kind="ExternalInput").ap()
    W2 = nc.dram_tensor("W2", [D, U], F32, kind="ExternalInput").ap()
    b2 = nc.dram_tensor("b2", [U, 1], F32, kind="ExternalInput").ap()
    V = nc.dram_tensor("V", [U, 1], F32, kind="ExternalInput").ap()
    ctx_out = nc.dram_tensor("ctx", [BPC, D], F32, kind="ExternalOutput").ap()
    aw_out = nc.dram_tensor("aw", [BPC, S], F32, kind="ExternalOutput").ap()
    # score-row bounce buffer (columnarization); ignored by the host
    srows = nc.dram_tensor("srows", [BPC, NM, 512], F32, kind="ExternalOutput").ap()

    with tile.TileContext(nc) as tc:
        with (
            tc.tile_pool(name="const", bufs=1) as const,
            tc.tile_pool(name="vh", bufs=24) as vh_pool,
            tc.tile_pool(name="vt", bufs=4) as vt_pool,
            tc.tile_pool(name="tt", bufs=4) as tt_pool,
            tc.tile_pool(name="small", bufs=2) as small,
            tc.tile_pool(name="vtp", bufs=2, space="PSUM") as vtpsum,
            tc.tile_pool(name="ppp", bufs=2, space="PSUM") as ppsum,
            tc.tile_pool(name="srp", bufs=1, space="PSUM") as srpsum,
            tc.tile_pool(name="cxp", bufs=1, space="PSUM") as ctxpsum,
        ):
            # ---- constants ----
            W1sb = const.tile([128, NK, U], F32)
            nc.sync.dma_start(out=W1sb, in_=W1.rearrange("(k p) u -> p k u", p=128))
            W2h = const.tile([128, NK, U], F16)
            nc.gpsimd.dma_start(out=W2h, in_=W2.rearrange("(k p) u -> p k u", p=128))
            b1sb = const.tile([128, 1], F32)
            nc.sync.dma_start(out=b1sb, in_=b1)
            b2sb = const.tile([128, 1], F32)
            nc.sync.dma_start(out=b2sb, in_=b2)
            Vsb = const.tile([128, 1], F32)
            nc.sync.dma_start(out=Vsb, in_=V)
            Vh = const.tile([128, 1], F16)
            nc.vector.tensor_copy(Vh, Vsb)
            ident = const.tile([128, 128], F32)
            make_identity(nc, ident)
            identh = const.tile([128, 128], F16)
            nc.vector.tensor_copy(identh, ident)
            ones = const.tile([128, 128], F32)
            nc.vector.memset(ones, 1.0)

            # ---- q_proj (per core, all 4 batches at once) ----
            qT = const.tile([128, NK, BPC], F32)
            for k in range(NK):
                nc.gpsimd.dma_start(
                    out=qT[:, k, :],
                    in_=query[:, k * 128 : (k + 1) * 128].rearrange("b p -> p b"),
                )
            qp_t = ppsum.tile([128, 512], F32, tag="pp")
            qp = qp_t[:, :BPC]
            for k in range(NK):
                nc.tensor.matmul(
                    qp, W1sb[:, k, :], qT[:, k, :], start=(k == 0), stop=(k == NK - 1)
                )
            qpT = const.tile([128, BPC], F32)
            # q_proj + b1 + b2 (fold both biases into the tanh bias)
            nc.vector.tensor_scalar_add(qpT, qp, b1sb)
            nc.vector.tensor_scalar_add(qpT, qpT, b2sb)

            # ---- batches (software-pipelined: ctx/aw of batch b-1 are
            # emitted between the score phase and softmax of batch b, so the
            # PE stream never stalls on the softmax serial chain) ----
            state = {}

            def score_phase(b):
                vb = []
                for q in range(8):
                    t = vh_pool.tile([128, 4, D], F16, tag="vh")
                    nc.gpsimd.dma_start(
                        out=t,
                        in_=values[b, q * 512 : (q + 1) * 512, :].rearrange(
                            "(j p) d -> p j d", p=128
                        ),
                    )
                    vb.append(t)

                row_writes = []
                for m in range(NM):
                    qtile = vb[m]
                    jbase = 0
                    vtp = vtpsum.tile([128, NK, 512], F16)
                    for k in range(NK):
                        for j4 in range(4):
                            nc.tensor.transpose(
                                vtp[:, k, j4 * 128 : (j4 + 1) * 128],
                                qtile[:, j4, k * 128 : (k + 1) * 128],
                                identh,
                            )
                    vt = vt_pool.tile([128, NK, 512], F16)
                    for k in range(NK):
                        if k < 2:
                            nc.vector.tensor_copy(vt[:, k, :], vtp[:, k, :])
                        else:
                            nc.scalar.copy(vt[:, k, :], vtp[:, k, :])
                    pp = ppsum.tile([128, 512], F32, tag="pp")
                    for k in range(NK):
                        nc.tensor.matmul(
                            pp,
                            W2h[:, k, :],
                            vt[:, k, :],
                            start=(k == 0),
                            stop=(k == NK - 1),
                        )
                    tT = tt_pool.tile([128, 512], F16)
                    nc.scalar.activation(
                        tT, pp, AF.Tanh, bias=qpT[:, b : b + 1], scale=1.0
                    )
                    srow = srpsum.tile([128, 512], F32, tag="sr")
                    nc.tensor.matmul(srow[0:1, :], Vh, tT, start=True, stop=True)
                    row_sb = small.tile([1, 512], F32, tag="rowsb")
                    nc.vector.tensor_copy(row_sb, srow[0:1, :])
                    w_ins = nc.sync.dma_start(out=srows[b, m : m + 1, :], in_=row_sb)
                    row_writes.append(w_ins)
                state[b] = {"vb": vb, "row_writes": row_writes}

            def tail_phase(b):
                st = state[b]
                vb = st["vb"]
                e_sb = small.tile([128, NJ], F32, tag="esb")
                rss = []
                cp = ctxpsum.tile([128, 512], F32)
                for h in range(2):
                    rows32 = small.tile([16, 128], F32, tag=f"rows32{h}")
                    r_ins = nc.sync.dma_start(
                        out=rows32,
                        in_=srows[b, 4 * h : 4 * h + 4].rearrange(
                            "a (c f) -> (a c) f", f=128
                        ),
                    )
                    for w_ins in st["row_writes"][4 * h : 4 * h + 4]:
                        add_dep_helper(r_ins.ins, w_ins.ins, reason="scratch RAW")
                    sc_p = srpsum.tile([128, 512], F32, tag="sr")
                    nc.tensor.matmul(
                        sc_p[:, :16], rows32, ident[:16, :16], is_transpose=True,
                        start=True, stop=True,
                    )
                    rs = small.tile([128, 1], F32, tag=f"rs{h}")
                    nc.scalar.activation(
                        e_sb[:, 16 * h : 16 * h + 16], sc_p[:, :16], AF.Exp,
                        accum_out=rs,
                    )
                    rss.append(rs)
                    e16 = small.tile([128, 16], F16, tag=f"e16{h}")
                    nc.vector.tensor_copy(e16, e_sb[:, 16 * h : 16 * h + 16])
                    # context half (unnormalized e; scaled by 1/Z at the end)
                    for jj in range(16):
                        j = 16 * h + jj
                        nc.tensor.matmul(
                            cp[0:1, :],
                            e16[:, jj : jj + 1],
                            vb[j // 4][:, j % 4, :],
                            start=(j == 0),
                            stop=(j == NJ - 1),
                        )
                mp = ppsum.tile([128, 512], F32, tag="pp")
                nc.tensor.matmul(mp[0:1, 0:1], rss[0], ones[:, 0:1], start=True, stop=False)
                nc.tensor.matmul(mp[0:1, 0:1], rss[1], ones[:, 0:1], start=False, stop=True)
                invZ = small.tile([1, 1], F32, tag="invZ")
                nc.vector.reciprocal(invZ, mp[0:1, 0:1])
                nc.tensor.matmul(mp[:, 4:5], ones[0:1, :], invZ, start=True, stop=True)
                inv_p = small.tile([128, 1], F32, tag="invp")
                nc.vector.tensor_copy(inv_p, mp[:, 4:5])
                ctx_sb = small.tile([1, 512], F32, tag="ctxsb")
                nc.vector.tensor_scalar_mul(ctx_sb, cp[0:1, :], invZ)
                nc.sync.dma_start(out=ctx_out[b : b + 1, :], in_=ctx_sb)
                wt_p = srpsum.tile([128, 512], F32, tag="sr")
                nc.tensor.transpose(wt_p[:NJ, :128], e_sb, ident)
                wt_sb = small.tile([NJ, 128], F32, tag="wtsb")
                nc.vector.tensor_scalar_mul(wt_sb, wt_p[:NJ, :128], inv_p[:NJ, :])
                nc.sync.dma_start(
                    out=aw_out[b].rearrange("(j f) -> j f", f=128), in_=wt_sb
                )
                del state[b]

            score_phase(0)
            score_phase(1)
            tail_phase(0)
            score_phase(2)
            tail_phase(1)
            score_phase(3)
            tail_phase(2)
            tail_phase(3)

    return nc


_NC_CACHE = {}


def kernel(query, values, W1, b1, W2, b2, V, bv):
    query = np.ascontiguousarray(np.asarray(query, dtype=np.float32))
    values = np.ascontiguousarray(np.asarray(values, dtype=np.float32))
    W1 = np.ascontiguousarray(np.asarray(W1, dtype=np.float32))
    b1 = np.asarray(b1, dtype=np.float32).reshape(U, 1)
    W2 = np.ascontiguousarray(np.asarray(W2, dtype=np.float32))
    b2 = np.asarray(b2, dtype=np.float32).reshape(U, 1)
    V = np.ascontiguousarray(np.asarray(V, dtype=np.float32).reshape(U, 1))

    if "nc" not in _NC_CACHE:
        _NC_CACHE["nc"] = build_kernel()
    nc = _NC_CACHE["nc"]

    in_maps = []
    for c in range(NCORES):
        sl = slice(c * BPC, (c + 1) * BPC)
        in_maps.append(
            {
                "query": np.ascontiguousarray(query[sl]),
                "values": np.ascontiguousarray(values[sl]),
                "W1": W1,
                "b1": b1,
                "W2": W2,
                "b2": b2,
                "V": V,
            }
        )

    res = run_bass_kernel_spmd(nc, in_maps, core_ids=list(range(NCORES)))
    ctx = np.concatenate([res.results[c]["ctx"] for c in range(NCORES)], axis=0)
    aw = np.concatenate([res.results[c]["aw"] for c in range(NCORES)], axis=0)
    return ctx, aw.reshape(B, S, 1)
